# revision 1
# baseline (speedup 1.0000x reference)
"""Trainium2 Bass kernel for nn_CircuitLayer (GNN message passing / KCL circuit).

res[b, n] = sum over edges e: (+i_e at des, -i_e at src),
i_e = a_e * tanh(w_e * (v_src - v_des) + b_e),  v = [0, x][node]

Strategy (node-parallel over 8 NeuronCores):
  - Node slots [0, 50176) split: NC i owns 6272 slots (8 Q7 cores x 784 nodes,
    28 tiles of 28 nodes each).
  - Every edge-endpoint incidence is routed to the (NC, core, tile) owning its
    node, sorted/grouped by node; sign folding: src-incidence w'=+w, a'=-a;
    des-incidence w'=-w, a'=+a; contribution c = a'*tanh(w'*(v_own-v_other)+b).
  - Per tile the device: ap_gathers v_other/v_own from a per-core compact table
    (distinct endpoints, int16-indexable), computes c on DVE/ACT (bf16),
    prefix-scans c (f32 state) and gathers per-node segment boundary sums.
  - Per-NC outputs are disjoint node ranges -> no collective needed.
"""

import numpy as np

B, N, E = 16, 50000, 1600000
NN = N + 1
NCS = 8
QC = 8
NPT = 28
TPC = 28
ROUNDS = 4
TPR = TPC // ROUNDS
NPC = NPT * TPC          # 784 nodes per core
NPNC = NPC * QC          # 6272 node slots per NC
MAX_CLEN = 32768 - 16

_cache = {}


def _pad16(n):
    return (n + 15) & ~15


def _bf16(x):
    x = np.ascontiguousarray(x, np.float32)
    u = x.view(np.uint32)
    r = ((u >> 16) & 1) + 0x7FFF
    return ((u + r) & 0xFFFF0000).view(np.float32)


def _wrap16(v):
    # (S,) -> (16, S//16): out[p, s] = v[s*16 + p]
    return v.reshape(-1, 16).T.copy()


def _preprocess(x, param, src_node, des_node):
    import ml_dtypes

    src = np.asarray(src_node).astype(np.int64)
    des = np.asarray(des_node).astype(np.int64)
    a, w, b = (np.asarray(param[i], np.float32) for i in range(3))

    own = np.concatenate([src, des])
    other = np.concatenate([des, src])
    wp = np.concatenate([w, -w])
    ap_ = np.concatenate([-a, a])
    bp = np.concatenate([b, b])

    order = np.argsort(own, kind="stable")
    own, other = own[order], other[order]
    wp, ap_, bp = wp[order], ap_[order], bp[order]

    cnt = np.bincount(own, minlength=NN).astype(np.int64)
    cstart = np.zeros(NN + 1, np.int64)
    np.cumsum(cnt, out=cstart[1:])

    # global tile capacity
    tile_tot = np.bincount(np.arange(NN) // NPT, weights=cnt,
                           minlength=(NCS * QC * TPC))
    CAP = _pad16(int(tile_tot.max()) + 1 + 16)
    assert CAP <= 4096, CAP

    aux = np.concatenate([np.zeros((B, 1), np.float32),
                          np.asarray(x, np.float32)], axis=1)

    # ---- per (nc, core, round): distinct endpoint lists ----
    dls = [[[None] * QC for _ in range(ROUNDS)] for _ in range(NCS)]
    clen_need = 0
    for nc in range(NCS):
        for r in range(ROUNDS):
            for k in range(QC):
                n0 = nc * NPNC + k * NPC + r * TPR * NPT
                n1 = min(n0 + TPR * NPT, NN)
                if n0 >= NN:
                    dls[nc][r][k] = np.empty(0, np.int64)
                    continue
                s, e = cstart[n0], cstart[n1]
                u = np.unique(np.concatenate([other[s:e], own[s:e]]))
                dls[nc][r][k] = u
                clen_need = max(clen_need, len(u))
    CLEN = _pad16(clen_need)
    assert CLEN <= MAX_CLEN, CLEN

    IDXW = CAP // 16
    per_nc = []
    for nc in range(NCS):
        ctab = np.zeros((ROUNDS, 128, CLEN), np.float32)
        idxs = np.zeros((TPC, 128, 2 * IDXW + 2), np.int16)
        prm = np.zeros((TPC, 128, 5 * CAP), np.float32)
        for r in range(ROUNDS):
            for k in range(QC):
                dl = dls[nc][r][k]
                if len(dl):
                    ctab[r, 16 * k:16 * k + 16, :len(dl)] = aux[:, dl]
                for ti in range(TPR):
                    t = r * TPR + ti
                    n0 = nc * NPNC + k * NPC + t * NPT
                    ob = np.zeros(CAP, np.int16)
                    nb = np.zeros(CAP, np.int16)
                    wrow = np.zeros(CAP, np.float32)
                    brow = np.zeros(CAP, np.float32)
                    arow = np.zeros(CAP, np.float32)
                    mrow = np.ones(CAP, np.float32)
                    mrow[0] = 0.0
                    vrow = np.zeros((16, CAP), np.float32)
                    cnts = np.zeros(NPT, np.int64)
                    if n0 < NN:
                        n1 = min(n0 + NPT, NN)
                        s, e = cstart[n0], cstart[n1]
                        m = e - s
                        assert m + 1 <= CAP
                        ob[1:1 + m] = np.searchsorted(dl, other[s:e])
                        nb[1:1 + m] = np.searchsorted(dl, own[s:e])
                        wrow[1:1 + m] = wp[s:e]
                        brow[1:1 + m] = bp[s:e]
                        arow[1:1 + m] = ap_[s:e]
                        cnts[:n1 - n0] = cnt[n0:n1]
                        if m:
                            o_sl = own[s:e]
                            starts = np.ones(m, bool)
                            starts[1:] = o_sl[1:] != o_sl[:-1]
                            spos = np.nonzero(starts)[0] + 1
                            mrow[spos] = 0.0
                            vrow[:, spos] = aux[:, o_sl[starts]]
                    ends = np.zeros(32, np.int16)
                    ends[:NPT] = np.cumsum(cnts).astype(np.int16)
                    sl = slice(16 * k, 16 * k + 16)
                    idxs[t, sl, 0:IDXW] = _wrap16(ob)
                    idxs[t, sl, IDXW:2 * IDXW] = _wrap16(nb)
                    idxs[t, sl, 2 * IDXW:] = _wrap16(ends)
                    prm[t, sl, 0:CAP] = wrow
                    prm[t, sl, CAP:2 * CAP] = brow
                    prm[t, sl, 2 * CAP:3 * CAP] = arow
                    prm[t, sl, 3 * CAP:4 * CAP] = mrow
                    prm[t, sl, 4 * CAP:5 * CAP] = vrow
        per_nc.append(dict(
            ctab=ctab,
            idxs=idxs,
            prm=_bf16(prm).astype(ml_dtypes.bfloat16),
        ))
    return dict(CAP=CAP, CLEN=CLEN), per_nc


def _build_program(CAP, CLEN, repeat=1):
    import sys
    if "/opt/trn_rl_repo" not in sys.path:
        sys.path.insert(0, "/opt/trn_rl_repo")
    from contextlib import ExitStack
    from concourse import bass, bacc, mybir, tile

    f32 = mybir.dt.float32
    bf16 = mybir.dt.bfloat16
    i16 = mybir.dt.int16
    Alu = mybir.AluOpType
    IDXW = CAP // 16

    nc = bacc.Bacc("TRN2", target_bir_lowering=False, debug=False,
                   num_devices=NCS)
    ctab_d = nc.dram_tensor("ctab_in", [ROUNDS, 128, CLEN], f32,
                            kind="ExternalInput")
    idxs_d = nc.dram_tensor("idxs_in", [TPC, 128, 2 * IDXW + 2], i16,
                            kind="ExternalInput")
    prm_d = nc.dram_tensor("prm_in", [TPC, 128, 5 * CAP], bf16,
                           kind="ExternalInput")
    out_d = nc.dram_tensor("res_out", [128, TPC * NPT], f32,
                           kind="ExternalOutput")

    with tile.TileContext(nc) as tc, ExitStack() as ctx:
        ctab_p = ctx.enter_context(tc.tile_pool(name="ctab", bufs=1))
        gat_p = ctx.enter_context(tc.tile_pool(name="gat", bufs=2))
        in_p = ctx.enter_context(tc.tile_pool(name="inp", bufs=2))
        zz_p = ctx.enter_context(tc.tile_pool(name="zz", bufs=2))
        p_p = ctx.enter_context(tc.tile_pool(name="pp", bufs=2))
        e_p = ctx.enter_context(tc.tile_pool(name="ee", bufs=2))
        res_p = ctx.enter_context(tc.tile_pool(name="res", bufs=1))

        res = res_p.tile([128, TPC * NPT], f32, tag="res")
        for _rep in range(repeat):
         for r in range(ROUNDS):
            ctab = ctab_p.tile([128, CLEN], f32, tag="ctab")
            nc.sync.dma_start(ctab[:], ctab_d.ap()[r])
            for ti in range(TPR):
                t = r * TPR + ti
                idx = in_p.tile([128, 2 * IDXW + 2], i16, tag="idx")
                nc.sync.dma_start(idx[:], idxs_d.ap()[t])
                prm = in_p.tile([128, 5 * CAP], bf16, tag="prm")
                nc.sync.dma_start(prm[:], prm_d.ap()[t])

                go = gat_p.tile([128, CAP], f32, tag="go")
                nc.gpsimd.ap_gather(go[:], ctab[:], idx[:, 0:IDXW],
                                    128, CLEN, 1, CAP)
                gn = gat_p.tile([128, CAP], f32, tag="gn")
                nc.vector.tensor_tensor_scan(gn[:], prm[:, 3 * CAP:4 * CAP],
                                             prm[:, 4 * CAP:5 * CAP], 0.0,
                                             Alu.mult, Alu.add)

                z1 = zz_p.tile([128, CAP], bf16, tag="zz")
                nc.vector.tensor_tensor(z1[:], gn[:], go[:], Alu.subtract)
                z2 = zz_p.tile([128, CAP], bf16, tag="zz")
                nc.vector.tensor_tensor(z2[:], z1[:], prm[:, 0:CAP], Alu.mult)
                z3 = zz_p.tile([128, CAP], bf16, tag="zz")
                nc.vector.tensor_tensor(z3[:], z2[:], prm[:, CAP:2 * CAP],
                                        Alu.add)
                th = zz_p.tile([128, CAP], bf16, tag="zz")
                nc.scalar.activation(th[:], z3[:],
                                     mybir.ActivationFunctionType.Tanh)
                cc = zz_p.tile([128, CAP], bf16, tag="zz")
                nc.vector.tensor_tensor(cc[:], th[:], prm[:, 2 * CAP:3 * CAP],
                                        Alu.mult)
                P = p_p.tile([128, CAP], f32, tag="P")
                nc.vector.tensor_tensor_scan(P[:], cc[:], cc[:], 0.0,
                                             Alu.add, Alu.bypass)
                Eb = e_p.tile([128, 48], f32, tag="Eb")
                nc.vector.memset(Eb[:, 0:1], 0.0)
                nc.gpsimd.ap_gather(Eb[:, 1:33], P[:],
                                    idx[:, 2 * IDXW:2 * IDXW + 2],
                                    128, CAP, 1, 32)
                nc.vector.tensor_tensor(res[:, t * NPT:(t + 1) * NPT],
                                        Eb[:, 1:1 + NPT], Eb[:, 0:NPT],
                                        Alu.subtract)
        nc.sync.dma_start(out_d.ap()[:], res[:])
    nc.compile()
    return nc


def kernel(**inputs) -> np.ndarray:
    import sys
    if "/opt/trn_rl_repo" not in sys.path:
        sys.path.insert(0, "/opt/trn_rl_repo")
    from concourse.bass_utils import run_bass_kernel_spmd

    x = np.asarray(inputs["x"], np.float32)
    param = np.asarray(inputs["param"], np.float32)
    meta, per_nc = _preprocess(x, param, inputs["src_node"],
                               inputs["des_node"])
    key = (meta["CAP"], meta["CLEN"])
    if key not in _cache:
        _cache[key] = _build_program(*key)
    nc = _cache[key]

    in_maps = [{"ctab_in": d["ctab"], "idxs_in": d["idxs"],
                "prm_in": d["prm"]} for d in per_nc]
    results = run_bass_kernel_spmd(nc, in_maps, list(range(NCS))).results

    full = np.zeros((B, NCS * NPNC), np.float32)
    for i, om in enumerate(results):
        o = om["res_out"]
        for k in range(QC):
            full[:, i * NPNC + k * NPC:i * NPNC + (k + 1) * NPC] = \
                o[16 * k:16 * k + 16]
    return np.ascontiguousarray(full[:, 1:NN])



# revision 2
# speedup vs baseline: 1.3917x; 1.3917x over previous
"""Trainium2 Bass kernel for nn_CircuitLayer (GNN message passing / KCL circuit).

res[b, n] = sum over edges e: (+i_e at des, -i_e at src),
i_e = a_e * tanh(w_e * (v_src - v_des) + b_e),  v = [0, x][node]

Design (node-parallel over 8 NeuronCores, batch along the free dim):
  - Each edge endpoint ("incidence") is routed to the node that owns it, with
    sign folding: src-incidence w'=+w, a'=-a; des-incidence w'=-w, a'=+a, so
    the node's contribution is c = a'*tanh(w'*(v_own - v_other) + b).
  - Nodes are assigned (descending degree, lightest-slot-first rounds) to
    NCS*C*128 slots so every slot carries ~equal incidence count J.
    Slot -> (nc, chunk, partition); outputs of different NCs are disjoint.
  - Per slot the free dim is [16 batch blocks x J incidences]; the host ships
    per-incidence v_own / v_other (bf16, batch-dependent, a pure gather of x)
    and per-incidence w', b, a', m rows (batch-independent, read on-device
    through 0-stride broadcast APs - no 16x replication in HBM).
  - Device per chunk: z = w'*(v_own - v_other) + b on DVE (bf16, 2x mode),
    tanh on the scalar/ACT engine, c = a'*th on DVE, then a segmented scan
    S (state = m*state + c, fp32 state, m=0 resets at node starts). Emission
    is software-pipelined so the DVE never stalls on the ACT tanh.
  - S (bf16) ships back to HBM; the host reads S at each node's last-incidence
    position (= that node's sum) and assembles the output. No gpsimd ops, no
    device-side gathers, no collectives (node ranges are disjoint across NCs).
"""

import numpy as np

B, N, E = 16, 50000, 1600000
NN = N + 1
NCS = 8
C = 6                      # chunks per NC
P = 128
S_TOT = NCS * C * P        # total slots

_cache = {}


def _pad16(n):
    return (n + 15) & ~15


def _preprocess(x, param, src_node, des_node):
    import ml_dtypes

    bf16 = ml_dtypes.bfloat16
    src = np.asarray(src_node).astype(np.int64)
    des = np.asarray(des_node).astype(np.int64)
    a, w, b = (np.asarray(param[i], np.float32) for i in range(3))

    own = np.concatenate([src, des])
    other = np.concatenate([des, src])
    wp = np.concatenate([w, -w])
    ap_ = np.concatenate([-a, a])
    bp = np.concatenate([b, b])

    keep = own != 0            # ground node (0) produces no output
    own, other = own[keep], other[keep]
    wp, ap_, bp = wp[keep], ap_[keep], bp[keep]

    order = np.argsort(own, kind="stable")
    own, other = own[order], other[order]
    wp, ap_, bp = wp[order], ap_[order], bp[order]
    I = len(own)

    cnt = np.bincount(own, minlength=NN).astype(np.int64)
    cstart = np.zeros(NN + 1, np.int64)
    np.cumsum(cnt, out=cstart[1:])

    # assign nodes (desc degree) to S_TOT slots; each round sends the next
    # S_TOT heaviest nodes to the currently lightest slots.
    deg = cnt[1:NN]                      # node n -> deg[n-1]
    ord2 = np.argsort(-deg, kind="stable")
    slot_of = np.empty(N, np.int64)
    loads = np.zeros(S_TOT, np.int64)
    for r0 in range(0, N, S_TOT):
        blk = ord2[r0:r0 + S_TOT]
        rank = np.argsort(loads, kind="stable")
        sl = rank[:len(blk)]
        slot_of[blk] = sl
        loads[sl] += deg[blk]
    J = _pad16(int(loads.max()))
    J16 = 16 * J

    # nodes in slot-major order; within-slot offsets
    perm_nodes = np.argsort(slot_of, kind="stable")   # values: n-1
    deg_s = deg[perm_nodes]
    slot_s = slot_of[perm_nodes]
    ecs = np.cumsum(deg_s) - deg_s                    # exclusive cumsum
    first = np.ones(N, bool)
    first[1:] = slot_s[1:] != slot_s[:-1]
    base = np.zeros(S_TOT, np.int64)
    base[slot_s[first]] = ecs[first]
    off = ecs - base[slot_s]                          # within-slot offset

    # ragged-range concat: incidence PERM (padded-dest ordering)
    starts = cstart[perm_nodes + 1]
    intra = np.arange(I, dtype=np.int64) - np.repeat(ecs, deg_s)
    PERM = np.repeat(starts, deg_s) + intra
    dest = np.repeat(slot_s * J + off, deg_s) + intra

    own_p = np.zeros(S_TOT * J, np.int64)
    oth_p = np.zeros(S_TOT * J, np.int64)
    wp_p = np.zeros(S_TOT * J, np.float32)
    ap_p = np.zeros(S_TOT * J, np.float32)   # pad a'=0 -> c=0
    bp_p = np.zeros(S_TOT * J, np.float32)
    m_p = np.ones(S_TOT * J, np.float32)
    own_p[dest] = own[PERM]
    oth_p[dest] = other[PERM]
    wp_p[dest] = wp[PERM]
    ap_p[dest] = ap_[PERM]
    bp_p[dest] = bp[PERM]
    nz = deg_s > 0
    m_p[(slot_s * J + off)[nz]] = 0.0                 # reset at node starts

    aux = np.concatenate([np.zeros((B, 1), np.float32),
                          np.asarray(x, np.float32)], axis=1)
    vo_all = aux[:, own_p].astype(bf16)               # (16, S_TOT*J)
    vd_all = aux[:, oth_p].astype(bf16)

    def to_nc(vall):
        v = vall.reshape(B, NCS, C, P, J).transpose(1, 2, 3, 0, 4)
        return np.ascontiguousarray(v).reshape(NCS, C, P, J16)

    vo_nc = to_nc(vo_all)
    vd_nc = to_nc(vd_all)

    prm = np.stack([wp_p, bp_p, ap_p, m_p])           # [4, S_TOT*J]
    prm_nc = np.ascontiguousarray(
        prm.reshape(4, NCS, C, P, J).transpose(1, 2, 3, 0, 4)
    ).reshape(NCS, C, P, 4 * J).astype(bf16)

    # node n's sum lives at s_out[nc, c, p, b*J + off + deg - 1]
    slot = slot_s
    extract = dict(perm_nodes=perm_nodes, nc_i=slot // (C * P),
                   c_i=(slot % (C * P)) // P, p_i=slot % P,
                   endj=off + deg_s - 1, nz=nz)

    per_nc = [dict(vo=vo_nc[i], vd=vd_nc[i], prm=prm_nc[i])
              for i in range(NCS)]
    return dict(J=J, extract=extract), per_nc


def _build_program(J, repeat=1):
    import sys
    if "/opt/trn_rl_repo" not in sys.path:
        sys.path.insert(0, "/opt/trn_rl_repo")
    from contextlib import ExitStack
    from concourse import bacc, mybir, tile

    bf16 = mybir.dt.bfloat16
    Alu = mybir.AluOpType
    J16 = 16 * J

    nc = bacc.Bacc("TRN2", target_bir_lowering=False, debug=False,
                   num_devices=NCS)
    vo_d = nc.dram_tensor("vo_in", [C, P, J16], bf16, kind="ExternalInput")
    vd_d = nc.dram_tensor("vd_in", [C, P, J16], bf16, kind="ExternalInput")
    prm_d = nc.dram_tensor("prm_in", [C, P, 4 * J], bf16,
                           kind="ExternalInput")
    s_d = nc.dram_tensor("s_out", [C, P, J16], bf16, kind="ExternalOutput")

    def b3(ap2):
        return ap2.rearrange("p (b j) -> p b j", b=16)

    with tile.TileContext(nc) as tc, ExitStack() as ctx:
        in_p = ctx.enter_context(tc.tile_pool(name="inp", bufs=2))
        m_p = ctx.enter_context(tc.tile_pool(name="mm", bufs=1))
        z_p = ctx.enter_context(tc.tile_pool(name="zz", bufs=2))
        t_p = ctx.enter_context(tc.tile_pool(name="tt", bufs=2))
        s_p = ctx.enter_context(tc.tile_pool(name="ss", bufs=2))

        pending = []   # (chunk_idx, th_tile, prm_tile, m16_tile)

        def stage_b():
            c, th, prm, m16 = pending.pop(0)
            ab = prm[:, 2 * J:3 * J][:, None, :].broadcast_to([P, 16, J])
            cc = z_p.tile([P, J16], bf16, tag="z")
            nc.vector.tensor_tensor(b3(cc[:]), b3(th[:]), ab, Alu.mult)
            S = s_p.tile([P, J16], bf16, tag="S")
            nc.vector.tensor_tensor_scan(S[:], m16[:], cc[:], 0.0,
                                         Alu.mult, Alu.add)
            nc.sync.dma_start(s_d.ap()[c], S[:])

        for _rep in range(repeat):
            for c in range(C):
                vo = in_p.tile([P, J16], bf16, tag="vo")
                vd = in_p.tile([P, J16], bf16, tag="vd")
                prm = in_p.tile([P, 4 * J], bf16, tag="prm")
                nc.sync.dma_start(vo[:], vo_d.ap()[c])
                nc.sync.dma_start(vd[:], vd_d.ap()[c])
                nc.sync.dma_start(prm[:], prm_d.ap()[c])

                wb = prm[:, 0 * J:1 * J][:, None, :].broadcast_to([P, 16, J])
                bb = prm[:, 1 * J:2 * J][:, None, :].broadcast_to([P, 16, J])
                mb = prm[:, 3 * J:4 * J][:, None, :].broadcast_to([P, 16, J])

                z1 = z_p.tile([P, J16], bf16, tag="z")
                nc.vector.tensor_tensor(z1[:], vo[:], vd[:], Alu.subtract)
                z2 = z_p.tile([P, J16], bf16, tag="z")
                nc.vector.tensor_tensor(b3(z2[:]), b3(z1[:]), wb, Alu.mult)
                z3 = z_p.tile([P, J16], bf16, tag="z")
                nc.vector.tensor_tensor(b3(z3[:]), b3(z2[:]), bb, Alu.add)
                th = t_p.tile([P, J16], bf16, tag="th")
                nc.scalar.activation(th[:], z3[:],
                                     mybir.ActivationFunctionType.Tanh)
                m16 = m_p.tile([P, J16], bf16, tag="m16")
                nc.scalar.copy(b3(m16[:]), mb)
                pending.append((c, th, prm, m16))
                if len(pending) > 1:
                    stage_b()
            while pending:
                stage_b()
    nc.compile()
    return nc


def _assemble(meta, s_all):
    ex = meta["extract"]
    J = meta["J"]
    res = np.zeros((B, N), np.float32)
    nz = ex["nz"]
    pn = ex["perm_nodes"][nz]
    bidx = np.arange(B)[:, None]
    vals = s_all[ex["nc_i"][nz], ex["c_i"][nz], ex["p_i"][nz],
                 bidx * J + ex["endj"][nz]]
    res[:, pn] = vals
    return res


def kernel(**inputs) -> np.ndarray:
    import sys
    if "/opt/trn_rl_repo" not in sys.path:
        sys.path.insert(0, "/opt/trn_rl_repo")
    from concourse.bass_utils import run_bass_kernel_spmd

    x = np.asarray(inputs["x"], np.float32)
    param = np.asarray(inputs["param"], np.float32)
    meta, per_nc = _preprocess(x, param, inputs["src_node"],
                               inputs["des_node"])
    J = meta["J"]
    if J not in _cache:
        _cache[J] = _build_program(J)
    nc = _cache[J]

    in_maps = [{"vo_in": d["vo"], "vd_in": d["vd"], "prm_in": d["prm"]}
               for d in per_nc]
    results = run_bass_kernel_spmd(nc, in_maps, list(range(NCS))).results
    s_all = np.stack([np.asarray(r["s_out"], np.float32) for r in results])
    return _assemble(meta, s_all)
